# revision 26
# baseline (speedup 1.0000x reference)
"""AttentionPool2d Trainium2 kernel, 8-core batch-data-parallel.

Math (reference returns only query position 0):
  x' = x.flat + pos_sp (pre-added on host); posc = pos_m - mean(pos_sp)
  xf0 = mean_s(x') + posc    (mean-token input vector; written into x'
                              col 256 so the mean-token logit rides the
                              same 257-wide logits matmul)
  q0 = W_q @ xf0            (b_q folded into u0 = blkdiag(W_k)^T b_q/8)
  u_h = W_k_h^T (q0_h/8) + u0_h
  l = u^T [x' | xf0] ; w = softmax(l) ; w' = w_sp + w_m/256
  y = x'^T w'
  a0_h = W_v_h y_h + (W_v posc) wm_h     (b_v folded into b_c')
  out = w_c a0 + b_c'       (b_c' = b_c + w_c b_v, host-precomputed)

DMA: inputs host-packed so each partition line is one contiguous >=4KB
descriptor.  Stream order x' -> W_q -> W_k -> u0 -> xt' -> W_v -> W_c on
the sync queue; compute chases the stream (means split vector/scalar per
slab, q0/u right after their weights land, per-batch attention pipeline,
a0/out tail).
"""
import sys
sys.path.insert(0, "/opt/trn_rl_repo")
import numpy as np
import ml_dtypes
from contextlib import ExitStack

from concourse import bacc, tile, mybir
from concourse import masks
from concourse.bass_utils import run_bass_kernel_spmd

P = 128
B, C, S2, L = 64, 1024, 256, 257
NH, CHD = 16, 64
NCORE, BPC, CT = 8, 8, 8          # cores, batches/core, c-tiles
F32R = mybir.dt.float32r
F32 = mybir.dt.float32
BF16 = mybir.dt.bfloat16
FP8 = mybir.dt.float8e4
AF = mybir.ActivationFunctionType
SCALE2 = 1.0 / 8.0                 # (1/ch^0.25)^2 folded into q0blk
NBV = 6                            # batches/slab reduced on vector (rest scalar)


def _body(ctx: ExitStack, tc, d):
    nc = tc.nc
    const = ctx.enter_context(tc.tile_pool(name="const", bufs=1))
    xres = ctx.enter_context(tc.tile_pool(name="xres", bufs=1))
    xtp = ctx.enter_context(tc.tile_pool(name="xtp", bufs=1))
    wts = ctx.enter_context(tc.tile_pool(name="wts", bufs=1))
    work = ctx.enter_context(tc.tile_pool(name="work", bufs=3))
    acc = ctx.enter_context(tc.tile_pool(name="acc", bufs=1))
    ps = ctx.enter_context(tc.tile_pool(name="ps", bufs=2, space="PSUM"))
    ps1 = ctx.enter_context(tc.tile_pool(name="ps1", bufs=2, space="PSUM"))

    identf = const.tile([16, 16], F32)
    masks.make_identity(nc, identf[:])
    ident = const.tile([16, 16], F32R)
    nc.vector.tensor_copy(ident[:], identf[:, :])

    # ---- DMA issue order = stream order (sync queue) ----
    posc = wts.tile([P, 2, CT], F32)               # posc, b_c' (c-part)
    nc.sync.dma_start(posc[:], d["posc"].ap())
    xt = xres.tile([P, CT, BPC, L], BF16)          # x' (c-part): [p, j, b, 257]
    wvposc = wts.tile([1, C], BF16)                # W_v @ posc (needed late)
    for k in range(3):
        nc.sync.dma_start(xt[:, 2 * k:2 * k + 2], d["xall"].ap()[2 * k:2 * k + 2]
                          .rearrange("j p b s -> p j b s"))
    for j in range(6, 8):
        nc.sync.dma_start(xt[:, j], d["xall"].ap()[j])
    wqt = wts.tile([P, CT, C], FP8)                # W_q^T  (c-part, q)
    for h in range(2):
        nc.sync.dma_start(wqt[:, 4 * h:4 * h + 4],
                          d["wqt"].ap()[4 * h:4 * h + 4].rearrange(
                              "j p q -> p j q"))
    wk = wts.tile([P, CT, CT, P], FP8)             # W_k  [kp, j, t, ci]
    for h in range(2):
        nc.sync.dma_start(wk[:, 4 * h:4 * h + 4],
                          d["wk"].ap()[4 * h:4 * h + 4].rearrange(
                              "j p t c -> p j t c"))
    u0rep = wts.tile([P, CT * P], BF16)            # blkdiag(Wk)^T bq/8, b-bcast
    nc.sync.dma_start(u0rep[:], d["u0rep"].ap())
    xtt = xtp.tile([P, BPC, 2, C], BF16)           # xt' (s-part): [p, b, t, c]
    for k in range(4):
        nc.sync.dma_start(xtt[:, 2 * k:2 * k + 2], d["xtp"].ap()[2 * k:2 * k + 2]
                          .rearrange("b p t c -> p b t c"))
    wvt = wts.tile([P, CT, C], BF16)               # W_v^T (c-part, vch)
    for h in range(2):
        nc.sync.dma_start(wvt[:, 4 * h:4 * h + 4],
                          d["wvt"].ap()[4 * h:4 * h + 4].rearrange(
                              "j p q -> p j q"))
    wct = wts.tile([P, CT, C], BF16)               # w_c^T (vch-part, o)
    for h in range(2):
        nc.sync.dma_start(wct[:, 4 * h:4 * h + 4],
                          d["wct"].ap()[4 * h:4 * h + 4].rearrange(
                              "j p q -> p j q"))
    nc.sync.dma_start(wvposc[:], d["wvposc"].ap())

    # ---- stage A: means chase x' slabs; xf0 = mean + posc -> x' col 256 ----
    sums = acc.tile([P, CT, BPC], F32R)
    xf0 = acc.tile([P, CT * BPC], BF16)            # (c-part, (j, b))
    def meanslab(js):
        nj = len(js)
        padd = work.tile([P, nj, NBV, S2 // 2], BF16, tag=f"padd{nj}")
        nc.vector.tensor_add(padd[:], xt[:, js[0]:js[0] + nj, 0:NBV, 0:S2 // 2],
                             xt[:, js[0]:js[0] + nj, 0:NBV, S2 // 2:S2])
        nc.vector.reduce_sum(sums[:, js[0]:js[0] + nj, 0:NBV], padd[:],
                             axis=mybir.AxisListType.X)
        for j in js:
            for b in range(NBV, BPC):
                scr = work.tile([P, S2], F32R, tag="scr")
                nc.scalar.activation(scr[:], xt[:, j, b, 0:S2], AF.Copy,
                                     accum_out=sums[:, j, b:b + 1])
        for j in js:
            nc.vector.tensor_scalar(xf0[:, j * BPC:(j + 1) * BPC], sums[:, j],
                                    1.0 / S2, posc[:, 0, j:j + 1],
                                    op0=mybir.AluOpType.mult,
                                    op1=mybir.AluOpType.add)
            nc.scalar.activation(xt[:, j, :, S2], xf0[:, j * BPC:(j + 1) * BPC],
                                 AF.Copy)
    for k in range(3):
        meanslab([2 * k, 2 * k + 1])
    meanslab([6])
    meanslab([7])

    # ---- stage B: q0 (pure matmuls, j-outer chases wqt/xf0 slabs) ----
    # psum pre-zeroed by DVE so every matmul accumulates (start=False);
    # the zero-region group check is skipped -- one logical group.
    q0f = ps1.tile([P, 4 * P], F32, tag="seq")
    q0p = q0f[:, 0:CT * BPC]        # (q-part, (i, b))
    nc.vector.memset(q0p[:], 0.0)
    for j in range(CT):
        for i in range(CT):
            nc.tensor.matmul(q0p[:, i * BPC:(i + 1) * BPC],
                             wqt[:, j, i * P:(i + 1) * P],
                             xf0[:, j * BPC:(j + 1) * BPC],
                             start=False, stop=(j == CT - 1 and i == CT - 1),
                             skip_group_check=True)
    # block-diagonal q0/8 for the per-head W_k^T fold (2 strided copies)
    q0blk = acc.tile([P, CT * 16], BF16)
    nc.vector.memset(q0blk[:], 0.0)
    q0v = q0f[:, 0:64].rearrange("p (i c) -> p i c", i=CT)
    blkv = q0blk[:, :].rearrange("p (i c) -> p i c", i=CT)
    nc.vector.tensor_scalar_mul(blkv[0:64, :, 0:BPC], q0v[0:64], SCALE2)
    nc.vector.tensor_scalar_mul(blkv[64:P, :, BPC:16], q0v[64:P], SCALE2)

    # ---- stage C: u = blockdiag(W_k)^T q0blk + u0 (4-j packed psum) ----
    usb = acc.tile([P, CT * P], BF16)               # (c-part, (j, t2h, b))
    for k in range(2):
        up = ps1.tile([P, 4 * P], F32, tag="seq")
        for j in range(4 * k, 4 * k + 4):
            jo = (j - 4 * k) * P
            for t in range(CT):
                nc.tensor.matmul(up[:, jo + t * 16:jo + (t + 1) * 16],
                                 wk[:, j, t],
                                 q0blk[:, t * 16:(t + 1) * 16])
        nc.vector.tensor_add(usb[:, 4 * k * P:4 * (k + 1) * P], up[:, :],
                             u0rep[:, 4 * k * P:4 * (k + 1) * P])

    # ---- per-batch: logits (257-wide incl mean token), softmax, y ----
    wta = acc.tile([P, 3 * P], BF16)                # w'^T batched (s-part,(t,h,b))
    yall = acc.tile([P, BPC * P], BF16)             # y (c-part, (b, j, h))
    yv = yall[:, :].rearrange("p (b j h) -> p j h b", b=BPC, j=CT, h=16)
    for b in range(BPC):
        lg = ps.tile([16, L], F32, tag="lg")
        for j in range(CT):
            nc.tensor.matmul(lg[:, 0:L],
                             usb[:, j * P + b: (j + 1) * P: 8],
                             xt[:, j, b, :],
                             start=(j == 0), stop=(j == CT - 1))
        # softmax over 257
        mx = work.tile([16, 4], F32, tag="mx")
        nc.vector.reduce_max(mx[:, 0:1], lg[:, 0:L], axis=mybir.AxisListType.X,
                             negate=True)
        ex = work.tile([16, L], F32R, tag="ex")
        nc.scalar.activation(ex[:, :], lg[:, 0:L], AF.Exp, bias=mx[:, 0:1],
                             accum_out=mx[:, 1:2])
        nc.vector.reciprocal(mx[:, 2:3], mx[:, 1:2])
        # w' = (e_sp + e_m/256) * r ; wm = e_m * r
        wp = work.tile([16, L], F32R, tag="wp")
        nc.vector.tensor_scalar_mul(mx[:, 3:4], ex[:, S2:S2 + 1], 1.0 / S2)
        nc.vector.tensor_scalar(wp[:, 0:S2], ex[:, 0:S2], mx[:, 3:4], mx[:, 2:3],
                                op0=mybir.AluOpType.add,
                                op1=mybir.AluOpType.mult)
        nc.vector.tensor_scalar(wp[:, S2:L], ex[:, S2:L], mx[:, 2:3], None,
                                op0=mybir.AluOpType.mult)
        # transpose w' -> (s-part, h) chunks; third chunk = wm row
        wtp = ps.tile([P, 48], F32R, tag="wt")
        nc.tensor.transpose(wtp[:, 0:16], wp[:, 0:P], ident[:, :])
        nc.tensor.transpose(wtp[:, 16:32], wp[:, P:S2], ident[:, :])
        nc.tensor.transpose(wtp[0:1, 32:48], wp[:, S2:L], ident[:, :])
        nc.vector.tensor_copy(wta[:, b:P:8], wtp[:, 0:16])
        nc.scalar.activation(wta[:, P + b:2 * P:8], wtp[:, 16:32], AF.Copy)
        nc.vector.tensor_copy(wta[0:1, 2 * P + b:3 * P:8], wtp[0:1, 32:48])
        # y_x: stationary xt' tiles, moving w'^T
        yp = ps.tile([P, P], F32, tag="y")
        for j in range(CT):
            for t in range(2):
                nc.tensor.matmul(yp[:, j * 16:(j + 1) * 16],
                                 xtt[:, b, t, j * P:(j + 1) * P],
                                 wta[:, t * P + b:(t + 1) * P:8],
                                 start=(t == 0), stop=(t == 1))
        nc.vector.tensor_copy(yall[:, b * P:(b + 1) * P], yp[:, :])

    # ---- a0 = blockdiag(W_v) y + (W_v posc) wm ----
    wmrow = acc.tile([1, P], BF16)                  # single-writer wm row
    nc.vector.tensor_copy(wmrow[:], wta[0:1, 2 * P:3 * P])
    a0 = acc.tile([P, CT * BPC], BF16)              # (vch-part, (r, b))
    for r in range(CT):
        if r % 2 == 0:
            a0p = ps.tile([P, P], F32, tag="y")     # 4-bank rotation
        else:
            a0p = ps1.tile([P, 4 * P], F32, tag="seq")
        nc.tensor.matmul(a0p[:, 0:16],
                         wvposc[0:1, r * P:(r + 1) * P],
                         wmrow[0:1, 2 * r * 8: 2 * r * 8 + 16],
                         start=True, stop=False)
        for j in range(CT):
            nc.tensor.matmul(a0p[:, 0:16],
                             wvt[:, j, r * P:(r + 1) * P],
                             yv[:, j, 2 * r:2 * r + 2, :],
                             start=False, stop=(j == CT - 1))
        nc.vector.tensor_copy(a0[0:64, r * BPC:(r + 1) * BPC],
                              a0p[0:64, 0:BPC])
        nc.vector.tensor_copy(a0[64:P, r * BPC:(r + 1) * BPC],
                              a0p[64:P, BPC:16])

    # ---- out = w_c a0 + b_c'  (b_c' = b_c + w_c b_v, host-folded) ----
    # single-shot (i, r) partials, r innermost; one segmented reduce
    po = ps1.tile([P, 4 * P], F32, tag="seq")
    pov = po[:, :].rearrange("p (i b r) -> p i b r", i=CT, b=BPC)
    for r in range(CT):
        for i in range(CT):
            nc.tensor.matmul(pov[:, i, :, r],
                             wct[:, r, i * P:(i + 1) * P],
                             a0[:, r * BPC:(r + 1) * BPC],
                             start=True, stop=True)
    osb = acc.tile([P, CT * BPC], F32)
    ot = acc.tile([P, CT, BPC], F32R)
    nc.vector.reduce_sum(ot[:, :, :], pov[:, :, :, :],
                         axis=mybir.AxisListType.X)
    for i in range(CT):
        nc.vector.tensor_scalar_add(osb[:, i * BPC:(i + 1) * BPC],
                                    ot[:, i], posc[:, 1, i:i + 1])
    nc.sync.dma_start(d["out"].ap(), osb[:])


DEBUG = False
_CACHE = {}


def _get_nc():
    if "nc" in _CACHE:
        return _CACHE["nc"]
    nc = bacc.Bacc("TRN2", target_bir_lowering=False, debug=False,
                   num_devices=NCORE)
    d = {}
    d["xall"] = nc.dram_tensor("xall", [CT, P, BPC, L], BF16,
                               kind="ExternalInput")
    d["xtp"] = nc.dram_tensor("xtp", [BPC, P, 2, C], BF16,
                              kind="ExternalInput")
    d["posc"] = nc.dram_tensor("posc", [P, 2, CT], F32, kind="ExternalInput")
    d["wvposc"] = nc.dram_tensor("wvposc", [1, C], BF16, kind="ExternalInput")
    d["u0rep"] = nc.dram_tensor("u0rep", [P, CT * P], BF16,
                                kind="ExternalInput")
    d["wqt"] = nc.dram_tensor("wqt", [CT, P, C], FP8, kind="ExternalInput")
    d["wk"] = nc.dram_tensor("wk", [CT, P, CT, P], FP8, kind="ExternalInput")
    d["wvt"] = nc.dram_tensor("wvt", [CT, P, C], BF16, kind="ExternalInput")
    d["wct"] = nc.dram_tensor("wct", [CT, P, C], BF16, kind="ExternalInput")
    d["out"] = nc.dram_tensor("out", [P, CT * BPC], F32, kind="ExternalOutput")
    with tile.TileContext(nc) as tc, ExitStack() as ctx, \
            nc.allow_low_precision(reason="float32r tiles hold f32 bits"):
        _body(ctx, tc, d)
    nc.compile()
    _CACHE["nc"] = nc
    return nc


def _prep_maps(inputs):
    bf16 = ml_dtypes.bfloat16
    xf = inputs["x"].reshape(B, C, S2).astype(np.float32)
    pos = inputs["pos_emb"].astype(np.float32)
    pos_sp = pos[:, 1:]                              # (C, 256)
    posc = pos[:, 0] - pos_sp.mean(axis=1)           # (C,)
    xp = (xf + pos_sp[None]).astype(bf16)            # (B, C, 256)
    xtr = np.ascontiguousarray(
        xp.astype(np.float32).transpose(0, 2, 1)).astype(bf16)  # (B, 256, C)

    wqkv = inputs["w_qkv"].astype(np.float32)
    wq, wkm, wv = wqkv[0:C], wqkv[C:2 * C], wqkv[2 * C:]
    fp8 = ml_dtypes.float8_e4m3
    wqt = np.ascontiguousarray(wq.T.reshape(CT, P, C)).astype(fp8)
    wkp = np.ascontiguousarray(
        wkm.reshape(CT, P, CT, P).transpose(2, 1, 0, 3)).astype(fp8)
    wvt = np.ascontiguousarray(wv.T.reshape(CT, P, C)).astype(bf16)
    wct = np.ascontiguousarray(
        inputs["w_c"].astype(np.float32).T.reshape(CT, P, C)).astype(bf16)

    bqkv = inputs["b_qkv"].astype(np.float32)
    bq, bv = bqkv[0:C], bqkv[2 * C:3 * C]
    # u0 = blockdiag(W_k)^T b_q / 8, replicated over batch columns
    u0 = np.zeros((C, NH), np.float32)
    for h in range(NH):
        u0[:, h] = wkm[h * CHD:(h + 1) * CHD].T @ bq[h * CHD:(h + 1) * CHD]
    u0 /= 8.0
    u0rep = np.broadcast_to(
        u0.reshape(CT, P, NH, 1), (CT, P, NH, BPC)).transpose(1, 0, 2, 3)
    # b_c' = b_c + w_c @ b_v
    bcp = inputs["b_c"].astype(np.float32) + inputs["w_c"].astype(np.float32) @ bv
    poscp = np.empty((P, 2, CT), np.float32)
    poscp[:, 0] = posc.reshape(CT, P).T
    poscp[:, 1] = bcp.reshape(CT, P).T

    shared = dict(posc=poscp,
                  wvposc=np.ascontiguousarray((wv @ posc).astype(bf16)[None]),
                  u0rep=np.ascontiguousarray(
                      u0rep.reshape(P, CT * P).astype(bf16)),
                  wqt=wqt, wk=wkp, wvt=wvt, wct=wct)
    maps = []
    for c in range(NCORE):
        m = dict(shared)
        xc = xp[c * BPC:(c + 1) * BPC]               # (8, C, 256)
        xall = np.empty((CT, P, BPC, L), bf16)
        xall[:, :, :, 0:S2] = xc.reshape(BPC, CT, P, S2).transpose(1, 2, 0, 3)
        xall[:, :, :, S2] = 0
        m["xall"] = xall
        xtc = xtr[c * BPC:(c + 1) * BPC]             # (8, 256, C)
        m["xtp"] = np.ascontiguousarray(
            xtc.reshape(BPC, 2, P, C).transpose(0, 2, 1, 3))
        maps.append(m)
    return maps


def kernel(**inputs) -> np.ndarray:
    nc = _get_nc()
    maps = _prep_maps(inputs)
    res = run_bass_kernel_spmd(nc, maps, list(range(NCORE)))
    outs = []
    for c in range(NCORE):
        arr = res.results[c]["out"].reshape(P, CT, BPC)
        outs.append(arr.transpose(2, 1, 0).reshape(BPC, C))
    return np.concatenate(outs, axis=0).astype(np.float32)


if __name__ == "__main__":
    rng = np.random.default_rng(0)
    ins = {
        "x": rng.standard_normal((B, C, 16, 16), dtype=np.float32),
        "pos_emb": rng.standard_normal((C, L), dtype=np.float32) / 32,
        "w_qkv": rng.standard_normal((3 * C, C), dtype=np.float32) / 32,
        "b_qkv": rng.standard_normal((3 * C,), dtype=np.float32) * 0.1,
        "w_c": rng.standard_normal((C, C), dtype=np.float32) / 32,
        "b_c": rng.standard_normal((C,), dtype=np.float32) * 0.1,
    }
    o = kernel(**ins)
    print("out", o.shape, o.dtype, float(np.abs(o).mean()))
